# revision 17
# baseline (speedup 1.0000x reference)
"""Trainium2 Bass kernel for fused LN -> QKV -> (K^T V softmax) linear-attention -> out-proj + residual.

Algebraic restructure vs the direct formulation (kt_v is per-head 64x64 over
the whole sequence, so the K/V path funnels through small weight-side
products):

    xn   = (x - mu) / sigma                    (ln_g folded into the weights)
    G    = xn^T xn                             (1024x1024 Gram, contract tokens)
    ktv  = Wk^T G Wv        per head h: ktv_h = Wk_h^T G Wv_h    (linear in G)
    s    = softmax(ktv, axis=e)
    out  = (xn Wq) @ (blockdiag(s) Wout) + x   (q @ W2; q hides the AllReduce)

K and V activations are never materialized: G costs half the K,V projection
and ktv is tiny. The Q projection is kept (rather than folding Wq into a
single W3 = Wq S Wout) because it is collective-independent: issued right
after the ktv AllReduce starts, its ~55us of PE work fully hides the
collective's ~50us latency.

Precision split (the softmax logits are hypersensitive -- std ~64 -- so the
ktv path must stay f32): xn/G/A/Wk/Wv are f32r (1 cycle/row on the PE for
free-dim >= 256, same as bf16); the post-softmax chain (s, Wout, W2) and the
q-path (xnT, Wq, qT) are bf16 (~2e-3 effect each).

Sharding: data-parallel, 8 shards of 2048 tokens. Cores 2b, 2b+1 hold batch
element b; ktv partials (linear in G) are AllReduced pairwise, everything
else is local.
"""

import numpy as np

# Problem shapes (hardcoded per harness contract).
B, L, D = 4, 4096, 1024
H, HD = 16, 64
NCORES = 8
TOK = B * L // NCORES  # 2048 tokens per core
P = 128
NT = TOK // P  # 16 token tiles per core
NC_ = D // P  # 8 channel tiles
EPS = 1e-5


def _build(tc, nc, mybir, x_ap, x16_ap, wk_ap, wv_ap, wq_ap, wout_ap, out_ap,
           use_collective=True):
    from concourse.masks import make_identity

    f32 = mybir.dt.float32
    f32r = mybir.dt.float32r
    bf16 = mybir.dt.bfloat16
    AF = mybir.ActivationFunctionType
    OP = mybir.AluOpType

    def popen(name, bufs, space="SBUF"):
        cm = tc.tile_pool(name=name, bufs=bufs, space=space)
        return cm, cm.__enter__()

    def pclose(cm):
        cm.__exit__(None, None, None)

    consts_cm, consts = popen("consts", 1)
    smpool_cm, smpool = popen("smpool", 1)
    dram_cm, dram = popen("dram", 1, space="DRAM")
    xn_cm, xn_pool = popen("xn_pool", 1)

    ident32 = consts.tile([P, P], f32)
    make_identity(nc, ident32)
    identr = consts.tile([P, P], f32r)
    nc.vector.tensor_copy(out=identr, in_=ident32)
    ident16 = consts.tile([P, P], bf16)
    make_identity(nc, ident16)
    eps_t = consts.tile([P, 1], f32)
    nc.vector.memset(eps_t, EPS)
    zero_t = consts.tile([P, P], bf16)
    nc.vector.memset(zero_t, 0.0)

    # Warm-up collective: absorbs the mesh/staging setup cost so the real
    # ktv AllReduce later starts hot.
    warm_sb = consts.tile([P, 4], f32)
    nc.vector.memset(warm_sb, 0.0)
    warm_in = dram.tile([P, 4], f32, name="warm_in")
    warm_out = dram.tile([P, 4], f32, name="warm_out")
    nc.gpsimd.dma_start(out=warm_in, in_=warm_sb)
    if use_collective:
        nc.gpsimd.collective_compute(
            "AllReduce",
            mybir.AluOpType.add,
            ins=[warm_in.opt()],
            outs=[warm_out.opt()],
            replica_groups=[[0, 1], [2, 3], [4, 5], [6, 7]],
        )

    # Persistent f32r xn (token-major) for the Gram path.
    xn = [xn_pool.tile([P, D], f32r, tag=f"xn{i}", name=f"xn{i}")
          for i in range(NT)]

    # f32 K/V projection weights and the Gram matrix (both freed at the
    # collective kick, so the bf16 tail reuses their SBUF space).
    # DMA queue split: x tiles stream on sync; wk on vector, wv on scalar so
    # the first LN tiles are not stuck behind 16MB of weight descriptors.
    wkv_cm, wkv_pool = popen("wkv_pool", 1)
    gsb_cm, gsb_pool = popen("gsb_pool", 1)
    g_sb = [gsb_pool.tile([P, D], f32r, tag=f"g{i}", name=f"gsb{i}")
            for i in range(NC_)]
    wk = [wkv_pool.tile([P, D], f32r, tag=f"wk{i}", name=f"wk{i}")
          for i in range(NC_)]
    wv = [wkv_pool.tile([P, D], f32r, tag=f"wv{i}", name=f"wv{i}")
          for i in range(NC_)]
    for ct in range(NC_):
        nc.scalar.dma_start(out=wk[ct], in_=wk_ap[ct * P:(ct + 1) * P, :].bitcast(f32r))
        nc.gpsimd.dma_start(out=wv[ct], in_=wv_ap[ct * P:(ct + 1) * P, :].bitcast(f32r))

    # ---- Phase A: LN -> xn (f32) + Gram accumulation ----
    # G needs 16 PSUM banks; ping-pong two 3-bank tag sets (6 banks + mm's 2)
    # across 6 passes over the resident xn tiles so pass N+1 never waits on
    # pass N's drains.
    a_cm, a_pools = zip(*[popen("xpool", 3), popen("stpool", 4)])
    xpool, stpool = a_pools
    g_ps_cm, g_psum = popen("g_psum", 1, space="PSUM")

    def ln_tile(tt):
        # Stats read a bf16 copy of x (2x DVE rate; the ~2^-9 input rounding
        # perturbs mean/rstd by ~1e-4 relative -- negligible). The normalize
        # itself reads the f32 x, split across the Scalar and Pool engines.
        x_t = xpool.tile([P, D], f32, tag="x", name="x_t")
        nc.sync.dma_start(out=x_t, in_=x_ap[tt * P:(tt + 1) * P, :])
        x16 = xpool.tile([P, D], bf16, tag="x16", name="x16")
        nc.sync.dma_start(out=x16, in_=x16_ap[tt * P:(tt + 1) * P, :])
        stats = stpool.tile([P, 2, 6], f32, tag="stats", name="stats")
        nc.vector.bn_stats(out=stats[:, 0, :], in_=x16[:, 0:512])
        nc.vector.bn_stats(out=stats[:, 1, :], in_=x16[:, 512:1024])
        mv = stpool.tile([P, 2], f32, tag="mv", name="mv")
        nc.vector.bn_aggr(out=mv, in_=stats)
        sd = stpool.tile([P, 1], f32, tag="sd", name="sd")
        nc.scalar.activation(out=sd, in_=mv[:, 1:2], func=AF.Sqrt, bias=eps_t)
        rstd = stpool.tile([P, 1], f32, tag="rstd", name="rstd")
        nc.vector.reciprocal(out=rstd, in_=sd)
        nmr = stpool.tile([P, 1], f32, tag="nmr", name="nmr")
        nc.vector.tensor_scalar(out=nmr, in0=mv[:, 0:1], scalar1=rstd,
                                scalar2=-1.0, op0=OP.mult, op1=OP.mult)
        nc.scalar.activation(out=xn[tt][:, 0:512], in_=x_t[:, 0:512],
                             func=AF.Identity, scale=rstd, bias=nmr)
        nc.gpsimd.tensor_scalar(out=xn[tt][:, 512:1024], in0=x_t[:, 512:1024],
                                scalar1=mv[:, 0:1], scalar2=rstd,
                                op0=OP.subtract, op1=OP.mult)

    # G is symmetric: compute the top 4 block-rows in full (8 half-blocks,
    # all 8 PSUM banks, overlapped with LN) plus the diagonal quarter
    # (cb 4-7, right half), then mirror the lower-left quarter by
    # PE-transposing G[0:4, 512:1024].
    GROUPS = [[(cb, h) for cb in range(4) for h in range(2)],
              [(cb, 1) for cb in range(4, 8)]]
    for pi, group in enumerate(GROUPS):
        ps = {}
        for si, (cb, h) in enumerate(group):
            ps[(cb, h)] = g_psum.tile([P, 512], f32, tag=f"gp{si}",
                                      name=f"gps{cb}_{h}")
        for tt in range(NT):
            if pi == 0:
                ln_tile(tt)
            for (cb, h) in group:
                nc.tensor.matmul(ps[(cb, h)], xn[tt][:, cb * P:(cb + 1) * P],
                                 xn[tt][:, h * 512:(h + 1) * 512],
                                 start=(tt == 0), stop=(tt == NT - 1))
        for i, (cb, h) in enumerate(ps):
            if i % 2 == 0:
                nc.vector.tensor_copy(out=g_sb[cb][:, h * 512:(h + 1) * 512],
                                      in_=ps[(cb, h)])
            else:
                nc.scalar.copy(out=g_sb[cb][:, h * 512:(h + 1) * 512],
                               in_=ps[(cb, h)])

    pclose(g_ps_cm)
    for cm in reversed(a_cm):
        pclose(cm)

    # Mirror: g_sb[4+i][:, 0:512] (4 blocks each) = G[0:4, 512:1024]^T.
    trm_cm, trm_psum = popen("trm_psum", 2, space="PSUM")
    for i in range(4):
        trm = trm_psum.tile([P, 512], f32r, tag="trm", name="trm")
        for j in range(4):
            nc.tensor.transpose(
                trm[:, j * P:(j + 1) * P],
                g_sb[j][:, 512 + i * P:512 + (i + 1) * P], identr)
        if i % 2 == 0:
            nc.vector.tensor_copy(out=g_sb[4 + i][:, 0:512], in_=trm)
        else:
            nc.scalar.copy(out=g_sb[4 + i][:, 0:512], in_=trm)
    pclose(trm_cm)

    # ---- Phase A3 (fused): per c-block, A[cb] = G[:,cb]^T-chain @ Wv, then
    # immediately fold A[cb] into all 8 ktv pair-chains (contraction over cb).
    asb_cm, asb_pool = popen("asb_pool", 3)
    ktv_ps_cm, ktv_psum = popen("ktv_psum", 1, space="PSUM")
    a3m_cm, a3m_psum = popen("a3m_psum", 2, space="PSUM")
    a3x_cm, a3x_psum = popen("a3x_psum", 2, space="PSUM")

    # Two head pairs share one [P, 512] PSUM bank; only the very first matmul
    # into a bank sets start=True (marks the whole bank pending-zero, so the
    # second pair's first write is zero-initialized by the hardware).
    kt = [ktv_psum.tile([P, 512], f32, tag=f"kt{q}", name=f"kt{q}")
          for q in range(4)]
    for cb in range(NC_):
        mp0 = a3m_psum.tile([P, 512], f32, tag="mma", name="mp0")
        mp1 = a3x_psum.tile([P, 512], f32, tag="mmx", name="mp1")
        for ct in range(NC_):
            lhs = g_sb[ct][:, cb * P:(cb + 1) * P]
            nc.tensor.matmul(mp0, lhs, wv[ct][:, 0:512],
                             start=(ct == 0), stop=(ct == NC_ - 1))
            nc.tensor.matmul(mp1, lhs, wv[ct][:, 512:1024],
                             start=(ct == 0), stop=(ct == NC_ - 1))
        a_t = asb_pool.tile([P, D], f32r, tag="a", name="a_t")
        nc.vector.tensor_copy(out=a_t[:, 0:512], in_=mp0)
        nc.scalar.copy(out=a_t[:, 512:1024], in_=mp1)
        for q in range(4):
            for pr in range(2):
                p = 2 * q + pr
                nc.tensor.matmul(kt[q][:, pr * 256:(pr + 1) * 256],
                                 wk[cb][:, p * P:(p + 1) * P],
                                 a_t[:, (p // 2) * 256:(p // 2 + 1) * 256],
                                 start=(cb == 0 and pr == 0),
                                 stop=(cb == NC_ - 1 and pr == 1))

    # Stage ktv diag blocks: partition (h%2)*64+d, free (g=h//2, e).
    # Pair p sits in bank q=p//2 at column base (p%2)*256; within its
    # 256-wide quad slice head 2p is at offset (p%2)*128.
    stage = smpool.tile([P, 8, 64], f32, tag="sm864", name="stage")
    for p in range(NC_):
        q, pr = p // 2, p % 2
        off = pr * 256 + pr * 128
        nc.vector.tensor_copy(out=stage[0:64, p, :],
                              in_=kt[q][0:64, off:off + 64])
        nc.scalar.copy(out=stage[64:128, p, :],
                       in_=kt[q][64:128, off + 64:off + 128])

    # ---- Phase B: AllReduce ktv partials across the batch pair ----
    bounce_in = dram.tile([P, 512], f32, name="bounce_in")
    bounce_out = dram.tile([P, 512], f32, name="bounce_out")
    nc.gpsimd.dma_start(out=bounce_in, in_=stage.rearrange("p g e -> p (g e)"))
    if use_collective:
        nc.gpsimd.collective_compute(
            "AllReduce",
            mybir.AluOpType.add,
            ins=[bounce_in.opt()],
            outs=[bounce_out.opt()],
            replica_groups=[[0, 1], [2, 3], [4, 5], [6, 7]],
        )
    else:
        nc.gpsimd.dma_start(out=bounce_out, in_=bounce_in)
    kv_red = smpool.tile([P, 8, 64], f32, name="kv_red")
    nc.gpsimd.dma_start(out=kv_red.rearrange("p g e -> p (g e)"), in_=bounce_out)

    pclose(a3x_cm)
    pclose(a3m_cm)
    pclose(ktv_ps_cm)
    pclose(asb_cm)
    pclose(gsb_cm)
    pclose(wkv_cm)

    # ---- Transposes xn -> xnT, then Q projection (hide the collective) ----
    xnT_cm, xnT_pool = popen("xnT_pool", 1)
    qT_cm, qT_pool = popen("qT_pool", 1)
    wq_cm, wq_pool = popen("wq_pool", 1)
    tr_ps_cm, tr_psum = popen("tr_psum", 2, space="PSUM")
    qp_ps_cm, qp_psum = popen("qp_psum", 4, space="PSUM")

    xnT = [xnT_pool.tile([P, TOK], bf16, tag=f"xnT{i}", name=f"xnT{i}")
           for i in range(NC_)]
    qT = [qT_pool.tile([P, TOK], bf16, tag=f"qT{i}", name=f"qT{i}")
          for i in range(NC_)]
    wq = [wq_pool.tile([P, D], bf16, tag=f"wq{i}", name=f"wq{i}")
          for i in range(NC_)]
    for ct in range(NC_):
        nc.sync.dma_start(out=wq[ct], in_=wq_ap[ct * P:(ct + 1) * P, :])

    for ct in range(NC_):
        for tg in range(NT // 4):
            trt = tr_psum.tile([P, 512], f32r, tag="tr", name="trt")
            for i in range(4):
                tt = tg * 4 + i
                nc.tensor.transpose(trt[:, i * P:(i + 1) * P],
                                    xn[tt][:, ct * P:(ct + 1) * P], identr)
            if tg % 2 == 0:
                nc.vector.tensor_copy(out=xnT[ct][:, tg * 512:(tg + 1) * 512],
                                      in_=trt)
            else:
                nc.scalar.copy(out=xnT[ct][:, tg * 512:(tg + 1) * 512], in_=trt)

    # qT[jt] = Wq[:, jt-block]^T @ xn^T  (all bf16; runs during the AllReduce)
    for jt in range(NC_):
        for chunk in range(4):
            csl = slice(chunk * 512, (chunk + 1) * 512)
            qp = qp_psum.tile([P, 512], f32, tag="qp", name="qp")
            for ct in range(NC_):
                nc.tensor.matmul(qp, wq[ct][:, jt * P:(jt + 1) * P],
                                 xnT[ct][:, csl],
                                 start=(ct == 0), stop=(ct == NC_ - 1))
            if chunk % 2 == 0:
                nc.vector.tensor_copy(out=qT[jt][:, csl], in_=qp)
            else:
                nc.scalar.copy(out=qT[jt][:, csl], in_=qp)

    # ---- Phase C: softmax -> s^T (block-diag pairs) -> W2 ----
    negmax = smpool.tile([P, 8], f32, name="negmax")
    nc.vector.reduce_max(out=negmax, in_=kv_red, axis=mybir.AxisListType.X,
                         negate=True)
    s_t = smpool.tile([P, 8, 64], f32, tag="sm864b", name="s_t")
    sums = smpool.tile([P, 8], f32, name="sums")
    for g in range(8):
        nc.scalar.activation(out=s_t[:, g, :], in_=kv_red[:, g, :],
                             func=AF.Exp, bias=negmax[:, g:g + 1],
                             accum_out=sums[:, g:g + 1])
    rinv = smpool.tile([P, 8], f32, name="rinv")
    nc.vector.reciprocal(out=rinv, in_=sums)

    sblkT = smpool.tile([P, NC_ * P], bf16, tag="sbT", name="sblkT")
    for sg in range(2):
        trs = tr_psum.tile([P, 512], bf16, tag="trs", name="trs")
        for i in range(4):
            p = sg * 4 + i
            sblk = smpool.tile([P, P], bf16, tag=f"sbk{p % 2}", name="sblk")
            nc.vector.tensor_copy(out=sblk, in_=zero_t)
            nc.vector.tensor_scalar_mul(sblk[0:64, 0:64], s_t[0:64, p, :],
                                        rinv[0:64, p:p + 1])
            nc.vector.tensor_scalar_mul(sblk[64:128, 64:128], s_t[64:128, p, :],
                                        rinv[64:128, p:p + 1])
            nc.tensor.transpose(trs[:, i * P:(i + 1) * P], sblk, ident16)
        nc.scalar.copy(out=sblkT[:, sg * 512:(sg + 1) * 512], in_=trs)

    pclose(qp_ps_cm)
    pclose(tr_ps_cm)
    pclose(wq_cm)

    tail_cm, tail_pools = zip(*[
        popen("wout_pool", 1), popen("w2sb_pool", 1), popen("finm_psum", 6, "PSUM"),
        popen("xrpool", 3), popen("outpool", 3),
    ])
    wout_pool, w2sb_pool, finm_psum, xrpool, outpool = tail_pools

    wout = [wout_pool.tile([P, D], bf16, tag=f"wo{i}", name=f"wo{i}")
            for i in range(NC_)]
    for ct in range(NC_):
        nc.sync.dma_start(out=wout[ct], in_=wout_ap[ct * P:(ct + 1) * P, :])

    # W2 = blockdiag(s) @ Wout; pair p's rows live in wout tile p.
    w2_sb = [w2sb_pool.tile([P, D], bf16, tag=f"w2_{i}", name=f"w2_{i}")
             for i in range(NC_)]
    for p in range(NC_):
        mp0 = finm_psum.tile([P, 512], f32, tag="fm", name="mp0")
        mp1 = finm_psum.tile([P, 512], f32, tag="fm", name="mp1")
        sl = sblkT[:, p * P:(p + 1) * P]
        nc.tensor.matmul(mp0, sl, wout[p][:, 0:512], start=True, stop=True)
        nc.tensor.matmul(mp1, sl, wout[p][:, 512:1024], start=True, stop=True)
        nc.vector.tensor_copy(out=w2_sb[p][:, 0:512], in_=mp0)
        nc.scalar.copy(out=w2_sb[p][:, 512:1024], in_=mp1)

    # ---- Phase D: out = q @ W2 + x ----
    for tt in range(NT):
        tsl = slice(tt * P, (tt + 1) * P)
        xr = xrpool.tile([P, D], f32, tag="xr", name="xr")
        nc.sync.dma_start(out=xr, in_=x_ap[tsl, :])
        out_t = outpool.tile([P, D], f32, tag="out", name="out_t")
        mp0 = finm_psum.tile([P, 512], f32, tag="fm", name="mp0")
        mp1 = finm_psum.tile([P, 512], f32, tag="fm", name="mp1")
        for jt in range(NC_):
            lhs = qT[jt][:, tsl]
            nc.tensor.matmul(mp0, lhs, w2_sb[jt][:, 0:512],
                             start=(jt == 0), stop=(jt == NC_ - 1))
            nc.tensor.matmul(mp1, lhs, w2_sb[jt][:, 512:1024],
                             start=(jt == 0), stop=(jt == NC_ - 1))
        nc.vector.tensor_add(out=out_t[:, 0:512], in0=mp0, in1=xr[:, 0:512])
        nc.vector.tensor_add(out=out_t[:, 512:1024], in0=mp1, in1=xr[:, 512:1024])
        nc.sync.dma_start(out=out_ap[tsl, :], in_=out_t)

    for cm in reversed(tail_cm):
        pclose(cm)
    for cm in (qT_cm, xnT_cm, xn_cm, dram_cm, smpool_cm,
               consts_cm):
        pclose(cm)


def _make_program():
    """Build and compile the SPMD Bass program once."""
    import concourse.bass as bass  # noqa: F401
    import concourse.tile as tile
    from concourse import bacc, mybir

    nc = bacc.Bacc("TRN2", target_bir_lowering=False, debug=False,
                   num_devices=NCORES)
    f32 = mybir.dt.float32
    bf16 = mybir.dt.bfloat16
    x_d = nc.dram_tensor("x_shard", [TOK, D], f32, kind="ExternalInput").ap()
    x16_d = nc.dram_tensor("x16_shard", [TOK, D], bf16, kind="ExternalInput").ap()
    wk_d = nc.dram_tensor("w_k", [D, D], f32, kind="ExternalInput").ap()
    wv_d = nc.dram_tensor("w_v", [D, D], f32, kind="ExternalInput").ap()
    wq_d = nc.dram_tensor("w_q", [D, D], bf16, kind="ExternalInput").ap()
    wout_d = nc.dram_tensor("w_out", [D, D], bf16, kind="ExternalInput").ap()
    out_d = nc.dram_tensor("out_shard", [TOK, D], f32, kind="ExternalOutput").ap()

    with tile.TileContext(nc) as tc:
        _build(tc, nc, mybir, x_d, x16_d, wk_d, wv_d, wq_d, wout_d, out_d)
    nc.compile()
    return nc


_CACHED_NC = None


def _prepare_inputs(x, w_qkv, b_qkv, w_out, b_out, ln_g, ln_b):
    import ml_dtypes

    bf16 = ml_dtypes.bfloat16
    x = np.ascontiguousarray(np.asarray(x, dtype=np.float32))
    w_qkv = np.asarray(w_qkv, dtype=np.float32)
    b_qkv = np.asarray(b_qkv, dtype=np.float32)
    w_out = np.asarray(w_out, dtype=np.float32)
    b_out = np.asarray(b_out, dtype=np.float32)
    ln_g = np.asarray(ln_g, dtype=np.float32)
    ln_b = np.asarray(ln_b, dtype=np.float32)

    # Fold the LN affine into the QKV projection: xn@W + b with xn = z*g + lb
    # becomes z@(g[:,None]*W) + (b + lb@W).
    w_f = ln_g[:, None] * w_qkv
    b_eff = b_qkv + ln_b @ w_qkv
    if np.abs(b_eff).max() > 0 or np.abs(b_out).max() > 0:
        raise NotImplementedError("nonzero effective biases not supported")

    wq = np.ascontiguousarray(w_f[:, 0:D]).astype(bf16)
    wk = np.ascontiguousarray(w_f[:, D:2 * D])
    wv = np.ascontiguousarray(w_f[:, 2 * D:3 * D])
    wout = np.ascontiguousarray(w_out).astype(bf16)

    shards = x.reshape(NCORES, TOK, D)
    in_maps = [
        {"x_shard": np.ascontiguousarray(shards[c]),
         "x16_shard": np.ascontiguousarray(shards[c]).astype(bf16), "w_k": wk,
         "w_v": wv, "w_q": wq, "w_out": wout}
        for c in range(NCORES)
    ]
    return in_maps


def _run(inputs, trace=False):
    global _CACHED_NC
    from concourse.bass_utils import run_bass_kernel_spmd

    in_maps = _prepare_inputs(**inputs)
    if _CACHED_NC is None:
        _CACHED_NC = _make_program()
    res = run_bass_kernel_spmd(
        _CACHED_NC, in_maps, core_ids=list(range(NCORES)), trace=trace,
    )
    out = np.empty((B, L, D), dtype=np.float32)
    flat = out.reshape(NCORES, TOK, D)
    for c in range(NCORES):
        flat[c] = res.results[c]["out_shard"]
    return out, res


def kernel(**inputs):
    out, _ = _run(inputs, trace=False)
    return out


# revision 18
# speedup vs baseline: 1.2360x; 1.2360x over previous
"""Trainium2 Bass kernel for fused LN -> QKV -> (K^T V softmax) linear-attention -> out-proj + residual.

Algebraic restructure vs the direct formulation (kt_v is per-head 64x64 over
the whole sequence, so the K/V path funnels through small weight-side
products):

    xn   = (x - mu) / sigma                    (ln_g folded into the weights)
    G    = xn^T xn                             (1024x1024 Gram, contract tokens)
    ktv  = Wk^T G Wv        per head h: ktv_h = Wk_h^T G Wv_h    (linear in G)
    s    = softmax(ktv, axis=e)
    out  = (xn Wq) @ (blockdiag(s) Wout) + x   (q @ W2; q hides the AllReduce)

K and V activations are never materialized: G costs half the K,V projection
and ktv is tiny. The Q projection is kept (rather than folding Wq into a
single W3 = Wq S Wout) because it is collective-independent: issued right
after the ktv AllReduce starts, its ~55us of PE work fully hides the
collective's ~50us latency.

Precision split (the softmax logits are hypersensitive -- std ~64 -- so the
ktv path must stay f32): xn/G/A/Wk/Wv are f32r (1 cycle/row on the PE for
free-dim >= 256, same as bf16); the post-softmax chain (s, Wout, W2) and the
q-path (xnT, Wq, qT) are bf16 (~2e-3 effect each).

Sharding: data-parallel, 8 shards of 2048 tokens. Cores 2b, 2b+1 hold batch
element b; ktv partials (linear in G) are AllReduced pairwise, everything
else is local.
"""

import numpy as np

# Problem shapes (hardcoded per harness contract).
B, L, D = 4, 4096, 1024
H, HD = 16, 64
NCORES = 8
TOK = B * L // NCORES  # 2048 tokens per core
P = 128
NT = TOK // P  # 16 token tiles per core
NC_ = D // P  # 8 channel tiles
EPS = 1e-5


def _build(tc, nc, mybir, x_ap, x16_ap, wk_ap, wv_ap, wq_ap, wout_ap, out_ap,
           use_collective=True):
    from concourse.masks import make_identity

    f32 = mybir.dt.float32
    f32r = mybir.dt.float32r
    bf16 = mybir.dt.bfloat16
    AF = mybir.ActivationFunctionType
    OP = mybir.AluOpType

    def popen(name, bufs, space="SBUF"):
        cm = tc.tile_pool(name=name, bufs=bufs, space=space)
        return cm, cm.__enter__()

    def pclose(cm):
        cm.__exit__(None, None, None)

    consts_cm, consts = popen("consts", 1)
    smpool_cm, smpool = popen("smpool", 1)
    dram_cm, dram = popen("dram", 1, space="DRAM")
    xn_cm, xn_pool = popen("xn_pool", 1)

    ident32 = consts.tile([P, P], f32)
    make_identity(nc, ident32)
    identr = consts.tile([P, P], f32r)
    nc.vector.tensor_copy(out=identr, in_=ident32)
    ident16 = consts.tile([P, P], bf16)
    make_identity(nc, ident16)
    eps_t = consts.tile([P, 1], f32)
    nc.vector.memset(eps_t, EPS)
    zero_t = consts.tile([P, P], bf16)
    nc.vector.memset(zero_t, 0.0)

    # Warm-up collective: absorbs the mesh/staging setup cost so the real
    # ktv AllReduce later starts hot.
    warm_sb = consts.tile([P, 4], f32)
    nc.vector.memset(warm_sb, 0.0)
    warm_in = dram.tile([P, 4], f32, name="warm_in")
    warm_out = dram.tile([P, 4], f32, name="warm_out")
    nc.gpsimd.dma_start(out=warm_in, in_=warm_sb)
    if use_collective:
        nc.gpsimd.collective_compute(
            "AllReduce",
            mybir.AluOpType.add,
            ins=[warm_in.opt()],
            outs=[warm_out.opt()],
            replica_groups=[[0, 1], [2, 3], [4, 5], [6, 7]],
        )

    # Persistent f32r xn (token-major) for the Gram path.
    xn = [xn_pool.tile([P, D], f32r, tag=f"xn{i}", name=f"xn{i}")
          for i in range(NT)]

    # f32 K/V projection weights and the Gram matrix (both freed at the
    # collective kick, so the bf16 tail reuses their SBUF space).
    # DMA queue split: x tiles stream on sync; wk on vector, wv on scalar so
    # the first LN tiles are not stuck behind 16MB of weight descriptors.
    wkv_cm, wkv_pool = popen("wkv_pool", 1)
    gsb_cm, gsb_pool = popen("gsb_pool", 1)
    g_sb = [gsb_pool.tile([P, D], f32r, tag=f"g{i}", name=f"gsb{i}")
            for i in range(NC_)]
    wk = [wkv_pool.tile([P, D], f32r, tag=f"wk{i}", name=f"wk{i}")
          for i in range(NC_)]
    wv = [wkv_pool.tile([P, D], f32r, tag=f"wv{i}", name=f"wv{i}")
          for i in range(NC_)]
    for ct in range(NC_):
        nc.scalar.dma_start(out=wk[ct], in_=wk_ap[ct * P:(ct + 1) * P, :].bitcast(f32r))
        nc.gpsimd.dma_start(out=wv[ct], in_=wv_ap[ct * P:(ct + 1) * P, :].bitcast(f32r))

    # ---- Phase A: LN -> xn (f32) + Gram accumulation ----
    # G needs 16 PSUM banks; ping-pong two 3-bank tag sets (6 banks + mm's 2)
    # across 6 passes over the resident xn tiles so pass N+1 never waits on
    # pass N's drains.
    a_cm, a_pools = zip(*[popen("xpool", 3), popen("stpool", 4)])
    xpool, stpool = a_pools
    g_ps_cm, g_psum = popen("g_psum", 1, space="PSUM")

    def ln_tile(tt):
        # Stats read a bf16 copy of x (2x DVE rate; the ~2^-9 input rounding
        # perturbs mean/rstd by ~1e-4 relative -- negligible). The normalize
        # itself reads the f32 x, split across the Scalar and Pool engines.
        x_t = xpool.tile([P, D], f32, tag="x", name="x_t")
        nc.sync.dma_start(out=x_t, in_=x_ap[tt * P:(tt + 1) * P, :])
        x16 = xpool.tile([P, D], bf16, tag="x16", name="x16")
        nc.sync.dma_start(out=x16, in_=x16_ap[tt * P:(tt + 1) * P, :])
        stats = stpool.tile([P, 2, 6], f32, tag="stats", name="stats")
        nc.vector.bn_stats(out=stats[:, 0, :], in_=x16[:, 0:512])
        nc.vector.bn_stats(out=stats[:, 1, :], in_=x16[:, 512:1024])
        mv = stpool.tile([P, 2], f32, tag="mv", name="mv")
        nc.vector.bn_aggr(out=mv, in_=stats)
        sd = stpool.tile([P, 1], f32, tag="sd", name="sd")
        nc.scalar.activation(out=sd, in_=mv[:, 1:2], func=AF.Sqrt, bias=eps_t)
        rstd = stpool.tile([P, 1], f32, tag="rstd", name="rstd")
        nc.vector.reciprocal(out=rstd, in_=sd)
        nmr = stpool.tile([P, 1], f32, tag="nmr", name="nmr")
        nc.vector.tensor_scalar(out=nmr, in0=mv[:, 0:1], scalar1=rstd,
                                scalar2=-1.0, op0=OP.mult, op1=OP.mult)
        nc.scalar.activation(out=xn[tt], in_=x_t, func=AF.Identity, scale=rstd,
                             bias=nmr)

    # G is symmetric: compute the top 4 block-rows in full (8 half-blocks,
    # all 8 PSUM banks, overlapped with LN) plus the diagonal quarter
    # (cb 4-7, right half), then mirror the lower-left quarter by
    # PE-transposing G[0:4, 512:1024].
    GROUPS = [[(cb, h) for cb in range(4) for h in range(2)],
              [(cb, 1) for cb in range(4, 8)]]
    for pi, group in enumerate(GROUPS):
        ps = {}
        for si, (cb, h) in enumerate(group):
            ps[(cb, h)] = g_psum.tile([P, 512], f32, tag=f"gp{si}",
                                      name=f"gps{cb}_{h}")
        for tt in range(NT):
            if pi == 0:
                ln_tile(tt)
            for (cb, h) in group:
                nc.tensor.matmul(ps[(cb, h)], xn[tt][:, cb * P:(cb + 1) * P],
                                 xn[tt][:, h * 512:(h + 1) * 512],
                                 start=(tt == 0), stop=(tt == NT - 1))
        for i, (cb, h) in enumerate(ps):
            if i % 2 == 0:
                nc.vector.tensor_copy(out=g_sb[cb][:, h * 512:(h + 1) * 512],
                                      in_=ps[(cb, h)])
            else:
                nc.scalar.copy(out=g_sb[cb][:, h * 512:(h + 1) * 512],
                               in_=ps[(cb, h)])

    pclose(g_ps_cm)
    for cm in reversed(a_cm):
        pclose(cm)

    # Mirror: g_sb[4+i][:, 0:512] (4 blocks each) = G[0:4, 512:1024]^T.
    trm_cm, trm_psum = popen("trm_psum", 2, space="PSUM")
    for i in range(4):
        trm = trm_psum.tile([P, 512], f32r, tag="trm", name="trm")
        for j in range(4):
            nc.tensor.transpose(
                trm[:, j * P:(j + 1) * P],
                g_sb[j][:, 512 + i * P:512 + (i + 1) * P], identr)
        if i % 2 == 0:
            nc.vector.tensor_copy(out=g_sb[4 + i][:, 0:512], in_=trm)
        else:
            nc.scalar.copy(out=g_sb[4 + i][:, 0:512], in_=trm)
    pclose(trm_cm)

    # ---- Phase A3 (fused): per c-block, A[cb] = G[:,cb]^T-chain @ Wv, then
    # immediately fold A[cb] into all 8 ktv pair-chains (contraction over cb).
    asb_cm, asb_pool = popen("asb_pool", 3)
    ktv_ps_cm, ktv_psum = popen("ktv_psum", 1, space="PSUM")
    a3m_cm, a3m_psum = popen("a3m_psum", 2, space="PSUM")
    a3x_cm, a3x_psum = popen("a3x_psum", 2, space="PSUM")

    # Two head pairs share one [P, 512] PSUM bank; only the very first matmul
    # into a bank sets start=True (marks the whole bank pending-zero, so the
    # second pair's first write is zero-initialized by the hardware).
    kt = [ktv_psum.tile([P, 512], f32, tag=f"kt{q}", name=f"kt{q}")
          for q in range(4)]
    for cb in range(NC_):
        mp0 = a3m_psum.tile([P, 512], f32, tag="mma", name="mp0")
        mp1 = a3x_psum.tile([P, 512], f32, tag="mmx", name="mp1")
        for ct in range(NC_):
            lhs = g_sb[ct][:, cb * P:(cb + 1) * P]
            nc.tensor.matmul(mp0, lhs, wv[ct][:, 0:512],
                             start=(ct == 0), stop=(ct == NC_ - 1))
            nc.tensor.matmul(mp1, lhs, wv[ct][:, 512:1024],
                             start=(ct == 0), stop=(ct == NC_ - 1))
        a_t = asb_pool.tile([P, D], f32r, tag="a", name="a_t")
        nc.vector.tensor_copy(out=a_t[:, 0:512], in_=mp0)
        nc.scalar.copy(out=a_t[:, 512:1024], in_=mp1)
        for q in range(4):
            for pr in range(2):
                p = 2 * q + pr
                nc.tensor.matmul(kt[q][:, pr * 256:(pr + 1) * 256],
                                 wk[cb][:, p * P:(p + 1) * P],
                                 a_t[:, (p // 2) * 256:(p // 2 + 1) * 256],
                                 start=(cb == 0 and pr == 0),
                                 stop=(cb == NC_ - 1 and pr == 1))

    # Stage ktv diag blocks: partition (h%2)*64+d, free (g=h//2, e).
    # Pair p sits in bank q=p//2 at column base (p%2)*256; within its
    # 256-wide quad slice head 2p is at offset (p%2)*128.
    stage = smpool.tile([P, 8, 64], f32, tag="sm864", name="stage")
    for p in range(NC_):
        q, pr = p // 2, p % 2
        off = pr * 256 + pr * 128
        nc.vector.tensor_copy(out=stage[0:64, p, :],
                              in_=kt[q][0:64, off:off + 64])
        nc.scalar.copy(out=stage[64:128, p, :],
                       in_=kt[q][64:128, off + 64:off + 128])

    # ---- Phase B: AllReduce ktv partials across the batch pair ----
    bounce_in = dram.tile([P, 512], f32, name="bounce_in")
    bounce_out = dram.tile([P, 512], f32, name="bounce_out")
    nc.gpsimd.dma_start(out=bounce_in, in_=stage.rearrange("p g e -> p (g e)"))
    if use_collective:
        nc.gpsimd.collective_compute(
            "AllReduce",
            mybir.AluOpType.add,
            ins=[bounce_in.opt()],
            outs=[bounce_out.opt()],
            replica_groups=[[0, 1], [2, 3], [4, 5], [6, 7]],
        )
    else:
        nc.gpsimd.dma_start(out=bounce_out, in_=bounce_in)
    kv_red = smpool.tile([P, 8, 64], f32, name="kv_red")
    nc.gpsimd.dma_start(out=kv_red.rearrange("p g e -> p (g e)"), in_=bounce_out)

    pclose(a3x_cm)
    pclose(a3m_cm)
    pclose(ktv_ps_cm)
    pclose(asb_cm)
    pclose(gsb_cm)
    pclose(wkv_cm)

    # ---- Transposes xn -> xnT, then Q projection (hide the collective) ----
    xnT_cm, xnT_pool = popen("xnT_pool", 1)
    qT_cm, qT_pool = popen("qT_pool", 1)
    wq_cm, wq_pool = popen("wq_pool", 1)
    tr_ps_cm, tr_psum = popen("tr_psum", 2, space="PSUM")
    qp_ps_cm, qp_psum = popen("qp_psum", 4, space="PSUM")

    xnT = [xnT_pool.tile([P, TOK], bf16, tag=f"xnT{i}", name=f"xnT{i}")
           for i in range(NC_)]
    qT = [qT_pool.tile([P, TOK], bf16, tag=f"qT{i}", name=f"qT{i}")
          for i in range(NC_)]
    wq = [wq_pool.tile([P, D], bf16, tag=f"wq{i}", name=f"wq{i}")
          for i in range(NC_)]
    for ct in range(NC_):
        nc.sync.dma_start(out=wq[ct], in_=wq_ap[ct * P:(ct + 1) * P, :])

    for ct in range(NC_):
        for tg in range(NT // 4):
            trt = tr_psum.tile([P, 512], f32r, tag="tr", name="trt")
            for i in range(4):
                tt = tg * 4 + i
                nc.tensor.transpose(trt[:, i * P:(i + 1) * P],
                                    xn[tt][:, ct * P:(ct + 1) * P], identr)
            if tg % 2 == 0:
                nc.vector.tensor_copy(out=xnT[ct][:, tg * 512:(tg + 1) * 512],
                                      in_=trt)
            else:
                nc.scalar.copy(out=xnT[ct][:, tg * 512:(tg + 1) * 512], in_=trt)

    # qT[jt] = Wq[:, jt-block]^T @ xn^T  (all bf16; runs during the AllReduce)
    for jt in range(NC_):
        for chunk in range(4):
            csl = slice(chunk * 512, (chunk + 1) * 512)
            qp = qp_psum.tile([P, 512], f32, tag="qp", name="qp")
            for ct in range(NC_):
                nc.tensor.matmul(qp, wq[ct][:, jt * P:(jt + 1) * P],
                                 xnT[ct][:, csl],
                                 start=(ct == 0), stop=(ct == NC_ - 1))
            if chunk % 2 == 0:
                nc.vector.tensor_copy(out=qT[jt][:, csl], in_=qp)
            else:
                nc.scalar.copy(out=qT[jt][:, csl], in_=qp)

    # ---- Phase C: softmax -> s^T (block-diag pairs) -> W2 ----
    negmax = smpool.tile([P, 8], f32, name="negmax")
    nc.vector.reduce_max(out=negmax, in_=kv_red, axis=mybir.AxisListType.X,
                         negate=True)
    s_t = smpool.tile([P, 8, 64], f32, tag="sm864b", name="s_t")
    sums = smpool.tile([P, 8], f32, name="sums")
    for g in range(8):
        nc.scalar.activation(out=s_t[:, g, :], in_=kv_red[:, g, :],
                             func=AF.Exp, bias=negmax[:, g:g + 1],
                             accum_out=sums[:, g:g + 1])
    rinv = smpool.tile([P, 8], f32, name="rinv")
    nc.vector.reciprocal(out=rinv, in_=sums)

    sblkT = smpool.tile([P, NC_ * P], bf16, tag="sbT", name="sblkT")
    for sg in range(2):
        trs = tr_psum.tile([P, 512], bf16, tag="trs", name="trs")
        for i in range(4):
            p = sg * 4 + i
            sblk = smpool.tile([P, P], bf16, tag=f"sbk{p % 2}", name="sblk")
            nc.vector.tensor_copy(out=sblk, in_=zero_t)
            nc.vector.tensor_scalar_mul(sblk[0:64, 0:64], s_t[0:64, p, :],
                                        rinv[0:64, p:p + 1])
            nc.vector.tensor_scalar_mul(sblk[64:128, 64:128], s_t[64:128, p, :],
                                        rinv[64:128, p:p + 1])
            nc.tensor.transpose(trs[:, i * P:(i + 1) * P], sblk, ident16)
        nc.scalar.copy(out=sblkT[:, sg * 512:(sg + 1) * 512], in_=trs)

    pclose(qp_ps_cm)
    pclose(tr_ps_cm)
    pclose(wq_cm)

    tail_cm, tail_pools = zip(*[
        popen("wout_pool", 1), popen("w2sb_pool", 1), popen("finm_psum", 6, "PSUM"),
        popen("xrpool", 3), popen("outpool", 3),
    ])
    wout_pool, w2sb_pool, finm_psum, xrpool, outpool = tail_pools

    wout = [wout_pool.tile([P, D], bf16, tag=f"wo{i}", name=f"wo{i}")
            for i in range(NC_)]
    for ct in range(NC_):
        nc.sync.dma_start(out=wout[ct], in_=wout_ap[ct * P:(ct + 1) * P, :])

    # W2 = blockdiag(s) @ Wout; pair p's rows live in wout tile p.
    w2_sb = [w2sb_pool.tile([P, D], bf16, tag=f"w2_{i}", name=f"w2_{i}")
             for i in range(NC_)]
    for p in range(NC_):
        mp0 = finm_psum.tile([P, 512], f32, tag="fm", name="mp0")
        mp1 = finm_psum.tile([P, 512], f32, tag="fm", name="mp1")
        sl = sblkT[:, p * P:(p + 1) * P]
        nc.tensor.matmul(mp0, sl, wout[p][:, 0:512], start=True, stop=True)
        nc.tensor.matmul(mp1, sl, wout[p][:, 512:1024], start=True, stop=True)
        nc.vector.tensor_copy(out=w2_sb[p][:, 0:512], in_=mp0)
        nc.scalar.copy(out=w2_sb[p][:, 512:1024], in_=mp1)

    # ---- Phase D: out = q @ W2 + x ----
    for tt in range(NT):
        tsl = slice(tt * P, (tt + 1) * P)
        xr = xrpool.tile([P, D], f32, tag="xr", name="xr")
        nc.sync.dma_start(out=xr, in_=x_ap[tsl, :])
        out_t = outpool.tile([P, D], f32, tag="out", name="out_t")
        mp0 = finm_psum.tile([P, 512], f32, tag="fm", name="mp0")
        mp1 = finm_psum.tile([P, 512], f32, tag="fm", name="mp1")
        for jt in range(NC_):
            lhs = qT[jt][:, tsl]
            nc.tensor.matmul(mp0, lhs, w2_sb[jt][:, 0:512],
                             start=(jt == 0), stop=(jt == NC_ - 1))
            nc.tensor.matmul(mp1, lhs, w2_sb[jt][:, 512:1024],
                             start=(jt == 0), stop=(jt == NC_ - 1))
        nc.vector.tensor_add(out=out_t[:, 0:512], in0=mp0, in1=xr[:, 0:512])
        nc.vector.tensor_add(out=out_t[:, 512:1024], in0=mp1, in1=xr[:, 512:1024])
        nc.sync.dma_start(out=out_ap[tsl, :], in_=out_t)

    for cm in reversed(tail_cm):
        pclose(cm)
    for cm in (qT_cm, xnT_cm, xn_cm, dram_cm, smpool_cm,
               consts_cm):
        pclose(cm)


def _make_program():
    """Build and compile the SPMD Bass program once."""
    import concourse.bass as bass  # noqa: F401
    import concourse.tile as tile
    from concourse import bacc, mybir

    nc = bacc.Bacc("TRN2", target_bir_lowering=False, debug=False,
                   num_devices=NCORES)
    f32 = mybir.dt.float32
    bf16 = mybir.dt.bfloat16
    x_d = nc.dram_tensor("x_shard", [TOK, D], f32, kind="ExternalInput").ap()
    x16_d = nc.dram_tensor("x16_shard", [TOK, D], bf16, kind="ExternalInput").ap()
    wk_d = nc.dram_tensor("w_k", [D, D], f32, kind="ExternalInput").ap()
    wv_d = nc.dram_tensor("w_v", [D, D], f32, kind="ExternalInput").ap()
    wq_d = nc.dram_tensor("w_q", [D, D], bf16, kind="ExternalInput").ap()
    wout_d = nc.dram_tensor("w_out", [D, D], bf16, kind="ExternalInput").ap()
    out_d = nc.dram_tensor("out_shard", [TOK, D], f32, kind="ExternalOutput").ap()

    with tile.TileContext(nc) as tc:
        _build(tc, nc, mybir, x_d, x16_d, wk_d, wv_d, wq_d, wout_d, out_d)
    nc.compile()
    return nc


_CACHED_NC = None


def _prepare_inputs(x, w_qkv, b_qkv, w_out, b_out, ln_g, ln_b):
    import ml_dtypes

    bf16 = ml_dtypes.bfloat16
    x = np.ascontiguousarray(np.asarray(x, dtype=np.float32))
    w_qkv = np.asarray(w_qkv, dtype=np.float32)
    b_qkv = np.asarray(b_qkv, dtype=np.float32)
    w_out = np.asarray(w_out, dtype=np.float32)
    b_out = np.asarray(b_out, dtype=np.float32)
    ln_g = np.asarray(ln_g, dtype=np.float32)
    ln_b = np.asarray(ln_b, dtype=np.float32)

    # Fold the LN affine into the QKV projection: xn@W + b with xn = z*g + lb
    # becomes z@(g[:,None]*W) + (b + lb@W).
    w_f = ln_g[:, None] * w_qkv
    b_eff = b_qkv + ln_b @ w_qkv
    if np.abs(b_eff).max() > 0 or np.abs(b_out).max() > 0:
        raise NotImplementedError("nonzero effective biases not supported")

    wq = np.ascontiguousarray(w_f[:, 0:D]).astype(bf16)
    wk = np.ascontiguousarray(w_f[:, D:2 * D])
    wv = np.ascontiguousarray(w_f[:, 2 * D:3 * D])
    wout = np.ascontiguousarray(w_out).astype(bf16)

    shards = x.reshape(NCORES, TOK, D)
    in_maps = [
        {"x_shard": np.ascontiguousarray(shards[c]),
         "x16_shard": np.ascontiguousarray(shards[c]).astype(bf16), "w_k": wk,
         "w_v": wv, "w_q": wq, "w_out": wout}
        for c in range(NCORES)
    ]
    return in_maps


def _run(inputs, trace=False):
    global _CACHED_NC
    from concourse.bass_utils import run_bass_kernel_spmd

    in_maps = _prepare_inputs(**inputs)
    if _CACHED_NC is None:
        _CACHED_NC = _make_program()
    res = run_bass_kernel_spmd(
        _CACHED_NC, in_maps, core_ids=list(range(NCORES)), trace=trace,
    )
    out = np.empty((B, L, D), dtype=np.float32)
    flat = out.reshape(NCORES, TOK, D)
    for c in range(NCORES):
        flat[c] = res.results[c]["out_shard"]
    return out, res


def kernel(**inputs):
    out, _ = _run(inputs, trace=False)
    return out


# revision 19
# speedup vs baseline: 1.3310x; 1.0769x over previous
"""Trainium2 Bass kernel for fused LN -> QKV -> (K^T V softmax) linear-attention -> out-proj + residual.

Algebraic restructure vs the direct formulation (kt_v is per-head 64x64 over
the whole sequence, so the K/V path funnels through small weight-side
products):

    xn   = (x - mu) / sigma                    (ln_g folded into the weights)
    G    = xn^T xn                             (1024x1024 Gram, contract tokens)
    ktv  = Wk^T G Wv        per head h: ktv_h = Wk_h^T G Wv_h    (linear in G)
    s    = softmax(ktv, axis=e)
    out  = (xn Wq) @ (blockdiag(s) Wout) + x   (q @ W2; q hides the AllReduce)

K and V activations are never materialized: G costs half the K,V projection
and ktv is tiny. The Q projection is kept (rather than folding Wq into a
single W3 = Wq S Wout) because it is collective-independent: issued right
after the ktv AllReduce starts, its ~55us of PE work fully hides the
collective's ~50us latency.

Precision split (the softmax logits are hypersensitive -- std ~64 -- so the
ktv path must stay f32): xn/G/A/Wk/Wv are f32r (1 cycle/row on the PE for
free-dim >= 256, same as bf16); the post-softmax chain (s, Wout, W2) and the
q-path (xnT, Wq, qT) are bf16 (~2e-3 effect each).

Sharding: data-parallel, 8 shards of 2048 tokens. Cores 2b, 2b+1 hold batch
element b; ktv partials (linear in G) are AllReduced pairwise, everything
else is local.
"""

import numpy as np

# Problem shapes (hardcoded per harness contract).
B, L, D = 4, 4096, 1024
H, HD = 16, 64
NCORES = 8
TOK = B * L // NCORES  # 2048 tokens per core
P = 128
NT = TOK // P  # 16 token tiles per core
NC_ = D // P  # 8 channel tiles
EPS = 1e-5


def _build(tc, nc, mybir, x_ap, wk_ap, wv_ap, wqT_ap, wout_ap, out_ap,
           use_collective=True):
    from concourse.masks import make_identity

    f32 = mybir.dt.float32
    f32r = mybir.dt.float32r
    bf16 = mybir.dt.bfloat16
    AF = mybir.ActivationFunctionType
    OP = mybir.AluOpType

    def popen(name, bufs, space="SBUF"):
        cm = tc.tile_pool(name=name, bufs=bufs, space=space)
        return cm, cm.__enter__()

    def pclose(cm):
        cm.__exit__(None, None, None)

    consts_cm, consts = popen("consts", 1)
    smpool_cm, smpool = popen("smpool", 1)
    dram_cm, dram = popen("dram", 1, space="DRAM")
    xn_cm, xn_pool = popen("xn_pool", 1)

    ident32 = consts.tile([P, P], f32)
    make_identity(nc, ident32)
    identr = consts.tile([P, P], f32r)
    nc.vector.tensor_copy(out=identr, in_=ident32)
    ident16 = consts.tile([P, P], bf16)
    make_identity(nc, ident16)
    eps_t = consts.tile([P, 1], f32)
    nc.vector.memset(eps_t, EPS)
    zero_t = consts.tile([P, P], bf16)
    nc.vector.memset(zero_t, 0.0)

    # Warm-up collective: absorbs the mesh/staging setup cost so the real
    # ktv AllReduce later starts hot.
    warm_sb = consts.tile([P, 4], f32)
    nc.vector.memset(warm_sb, 0.0)
    warm_in = dram.tile([P, 4], f32, name="warm_in")
    warm_out = dram.tile([P, 4], f32, name="warm_out")
    nc.gpsimd.dma_start(out=warm_in, in_=warm_sb)
    if use_collective:
        nc.gpsimd.collective_compute(
            "AllReduce",
            mybir.AluOpType.add,
            ins=[warm_in.opt()],
            outs=[warm_out.opt()],
            replica_groups=[[0, 1], [2, 3], [4, 5], [6, 7]],
        )

    # Persistent f32r xn (token-major) for the Gram path.
    xn = [xn_pool.tile([P, D], f32r, tag=f"xn{i}", name=f"xn{i}")
          for i in range(NT)]

    # f32 K/V projection weights and the Gram matrix (both freed at the
    # collective kick, so the bf16 tail reuses their SBUF space).
    # DMA queue split: x tiles stream on sync; wk on vector, wv on scalar so
    # the first LN tiles are not stuck behind 16MB of weight descriptors.
    wkv_cm, wkv_pool = popen("wkv_pool", 1)
    gsb_cm, gsb_pool = popen("gsb_pool", 1)
    g_sb = [gsb_pool.tile([P, D], f32r, tag=f"g{i}", name=f"gsb{i}")
            for i in range(NC_)]
    wk = [wkv_pool.tile([P, D], f32r, tag=f"wk{i}", name=f"wk{i}")
          for i in range(NC_)]
    wv = [wkv_pool.tile([P, D], f32r, tag=f"wv{i}", name=f"wv{i}")
          for i in range(NC_)]
    for ct in range(NC_):
        nc.scalar.dma_start(out=wk[ct], in_=wk_ap[ct * P:(ct + 1) * P, :].bitcast(f32r))
        nc.gpsimd.dma_start(out=wv[ct], in_=wv_ap[ct * P:(ct + 1) * P, :].bitcast(f32r))

    # ---- Phase A: LN -> xn (f32) + Gram accumulation ----
    # G needs 16 PSUM banks; ping-pong two 3-bank tag sets (6 banks + mm's 2)
    # across 6 passes over the resident xn tiles so pass N+1 never waits on
    # pass N's drains.
    a_cm, a_pools = zip(*[popen("xpool", 3), popen("stpool", 4)])
    xpool, stpool = a_pools
    g_ps_cm, g_psum = popen("g_psum", 1, space="PSUM")

    def ln_tile(tt):
        x_t = xpool.tile([P, D], f32, tag="x", name="x_t")
        nc.sync.dma_start(out=x_t, in_=x_ap[tt * P:(tt + 1) * P, :])
        stats = stpool.tile([P, 2, 6], f32, tag="stats", name="stats")
        nc.vector.bn_stats(out=stats[:, 0, :], in_=x_t[:, 0:512])
        nc.vector.bn_stats(out=stats[:, 1, :], in_=x_t[:, 512:1024])
        mv = stpool.tile([P, 2], f32, tag="mv", name="mv")
        nc.vector.bn_aggr(out=mv, in_=stats)
        sd = stpool.tile([P, 1], f32, tag="sd", name="sd")
        nc.scalar.activation(out=sd, in_=mv[:, 1:2], func=AF.Sqrt, bias=eps_t)
        rstd = stpool.tile([P, 1], f32, tag="rstd", name="rstd")
        nc.vector.reciprocal(out=rstd, in_=sd)
        nmr = stpool.tile([P, 1], f32, tag="nmr", name="nmr")
        nc.vector.tensor_scalar(out=nmr, in0=mv[:, 0:1], scalar1=rstd,
                                scalar2=-1.0, op0=OP.mult, op1=OP.mult)
        nc.scalar.activation(out=xn[tt], in_=x_t, func=AF.Identity, scale=rstd,
                             bias=nmr)

    # G is symmetric: compute the top 4 block-rows in full (8 half-blocks,
    # all 8 PSUM banks, overlapped with LN) plus the diagonal quarter
    # (cb 4-7, right half), then mirror the lower-left quarter by
    # PE-transposing G[0:4, 512:1024].
    GROUPS = [[(cb, h) for cb in range(4) for h in range(2)],
              [(cb, 1) for cb in range(4, 8)]]
    for pi, group in enumerate(GROUPS):
        ps = {}
        for si, (cb, h) in enumerate(group):
            ps[(cb, h)] = g_psum.tile([P, 512], f32, tag=f"gp{si}",
                                      name=f"gps{cb}_{h}")
        for tt in range(NT):
            if pi == 0:
                ln_tile(tt)
            for (cb, h) in group:
                nc.tensor.matmul(ps[(cb, h)], xn[tt][:, cb * P:(cb + 1) * P],
                                 xn[tt][:, h * 512:(h + 1) * 512],
                                 start=(tt == 0), stop=(tt == NT - 1))
        for i, (cb, h) in enumerate(ps):
            if i % 2 == 0:
                nc.vector.tensor_copy(out=g_sb[cb][:, h * 512:(h + 1) * 512],
                                      in_=ps[(cb, h)])
            else:
                nc.scalar.copy(out=g_sb[cb][:, h * 512:(h + 1) * 512],
                               in_=ps[(cb, h)])

    pclose(g_ps_cm)
    for cm in reversed(a_cm):
        pclose(cm)

    # Mirror: g_sb[4+i][:, 0:512] (4 blocks each) = G[0:4, 512:1024]^T.
    trm_cm, trm_psum = popen("trm_psum", 2, space="PSUM")
    for i in range(4):
        trm = trm_psum.tile([P, 512], f32r, tag="trm", name="trm")
        for j in range(4):
            nc.tensor.transpose(
                trm[:, j * P:(j + 1) * P],
                g_sb[j][:, 512 + i * P:512 + (i + 1) * P], identr)
        if i % 2 == 0:
            nc.vector.tensor_copy(out=g_sb[4 + i][:, 0:512], in_=trm)
        else:
            nc.scalar.copy(out=g_sb[4 + i][:, 0:512], in_=trm)
    pclose(trm_cm)

    # ---- Phase A3 (fused): per c-block, A[cb] = G[:,cb]^T-chain @ Wv, then
    # immediately fold A[cb] into all 8 ktv pair-chains (contraction over cb).
    asb_cm, asb_pool = popen("asb_pool", 3)
    ktv_ps_cm, ktv_psum = popen("ktv_psum", 1, space="PSUM")
    a3m_cm, a3m_psum = popen("a3m_psum", 2, space="PSUM")
    a3x_cm, a3x_psum = popen("a3x_psum", 2, space="PSUM")

    # Two head pairs share one [P, 512] PSUM bank; only the very first matmul
    # into a bank sets start=True (marks the whole bank pending-zero, so the
    # second pair's first write is zero-initialized by the hardware).
    kt = [ktv_psum.tile([P, 512], f32, tag=f"kt{q}", name=f"kt{q}")
          for q in range(4)]
    for cb in range(NC_):
        mp0 = a3m_psum.tile([P, 512], f32, tag="mma", name="mp0")
        mp1 = a3x_psum.tile([P, 512], f32, tag="mmx", name="mp1")
        for ct in range(NC_):
            lhs = g_sb[ct][:, cb * P:(cb + 1) * P]
            nc.tensor.matmul(mp0, lhs, wv[ct][:, 0:512],
                             start=(ct == 0), stop=(ct == NC_ - 1))
            nc.tensor.matmul(mp1, lhs, wv[ct][:, 512:1024],
                             start=(ct == 0), stop=(ct == NC_ - 1))
        a_t = asb_pool.tile([P, D], f32r, tag="a", name="a_t")
        nc.vector.tensor_copy(out=a_t[:, 0:512], in_=mp0)
        nc.scalar.copy(out=a_t[:, 512:1024], in_=mp1)
        for q in range(4):
            for pr in range(2):
                p = 2 * q + pr
                nc.tensor.matmul(kt[q][:, pr * 256:(pr + 1) * 256],
                                 wk[cb][:, p * P:(p + 1) * P],
                                 a_t[:, (p // 2) * 256:(p // 2 + 1) * 256],
                                 start=(cb == 0 and pr == 0),
                                 stop=(cb == NC_ - 1 and pr == 1))

    # Stage ktv diag blocks: partition (h%2)*64+d, free (g=h//2, e).
    # Pair p sits in bank q=p//2 at column base (p%2)*256; within its
    # 256-wide quad slice head 2p is at offset (p%2)*128.
    stage = smpool.tile([P, 8, 64], f32, tag="sm864", name="stage")
    for p in range(NC_):
        q, pr = p // 2, p % 2
        off = pr * 256 + pr * 128
        nc.vector.tensor_copy(out=stage[0:64, p, :],
                              in_=kt[q][0:64, off:off + 64])
        nc.scalar.copy(out=stage[64:128, p, :],
                       in_=kt[q][64:128, off + 64:off + 128])

    # ---- Phase B: AllReduce ktv partials across the batch pair ----
    bounce_in = dram.tile([P, 512], f32, name="bounce_in")
    bounce_out = dram.tile([P, 512], f32, name="bounce_out")
    nc.gpsimd.dma_start(out=bounce_in, in_=stage.rearrange("p g e -> p (g e)"))
    if use_collective:
        nc.gpsimd.collective_compute(
            "AllReduce",
            mybir.AluOpType.add,
            ins=[bounce_in.opt()],
            outs=[bounce_out.opt()],
            replica_groups=[[0, 1], [2, 3], [4, 5], [6, 7]],
        )
    else:
        nc.gpsimd.dma_start(out=bounce_out, in_=bounce_in)
    kv_red = smpool.tile([P, 8, 64], f32, name="kv_red")
    nc.gpsimd.dma_start(out=kv_red.rearrange("p g e -> p (g e)"), in_=bounce_out)

    pclose(a3x_cm)
    pclose(a3m_cm)
    pclose(ktv_ps_cm)
    pclose(asb_cm)
    pclose(gsb_cm)
    pclose(wkv_cm)

    # ---- Transposes xn -> xnT (hide the collective) ----
    xnT_cm, xnT_pool = popen("xnT_pool", 1)
    wqT_cm, wqT_pool = popen("wqT_pool", 1)
    tr_ps_cm, tr_psum = popen("tr_psum", 2, space="PSUM")

    xnT = [xnT_pool.tile([P, TOK], bf16, tag=f"xnT{i}", name=f"xnT{i}")
           for i in range(NC_)]
    wqT = [wqT_pool.tile([P, D], bf16, tag=f"wqT{i}", name=f"wqT{i}")
           for i in range(NC_)]
    for ct in range(NC_):
        nc.sync.dma_start(out=wqT[ct], in_=wqT_ap[ct * P:(ct + 1) * P, :])

    for ct in range(NC_):
        for tg in range(NT // 4):
            trt = tr_psum.tile([P, 512], f32r, tag="tr", name="trt")
            for i in range(4):
                tt = tg * 4 + i
                nc.tensor.transpose(trt[:, i * P:(i + 1) * P],
                                    xn[tt][:, ct * P:(ct + 1) * P], identr)
            if tg % 2 == 0:
                nc.vector.tensor_copy(out=xnT[ct][:, tg * 512:(tg + 1) * 512],
                                      in_=trt)
            else:
                nc.scalar.copy(out=xnT[ct][:, tg * 512:(tg + 1) * 512], in_=trt)

    # ---- Phase C: softmax -> s^T (block-diag pairs) -> W2 -> W3 ----
    negmax = smpool.tile([P, 8], f32, name="negmax")
    nc.vector.reduce_max(out=negmax, in_=kv_red, axis=mybir.AxisListType.X,
                         negate=True)
    s_t = smpool.tile([P, 8, 64], f32, tag="sm864b", name="s_t")
    sums = smpool.tile([P, 8], f32, name="sums")
    for g in range(8):
        nc.scalar.activation(out=s_t[:, g, :], in_=kv_red[:, g, :],
                             func=AF.Exp, bias=negmax[:, g:g + 1],
                             accum_out=sums[:, g:g + 1])
    rinv = smpool.tile([P, 8], f32, name="rinv")
    nc.vector.reciprocal(out=rinv, in_=sums)

    sblkT = smpool.tile([P, NC_ * P], bf16, tag="sbT", name="sblkT")
    for sg in range(2):
        trs = tr_psum.tile([P, 512], bf16, tag="trs", name="trs")
        for i in range(4):
            p = sg * 4 + i
            sblk = smpool.tile([P, P], bf16, tag=f"sbk{p % 2}", name="sblk")
            nc.vector.tensor_copy(out=sblk, in_=zero_t)
            nc.vector.tensor_scalar_mul(sblk[0:64, 0:64], s_t[0:64, p, :],
                                        rinv[0:64, p:p + 1])
            nc.vector.tensor_scalar_mul(sblk[64:128, 64:128], s_t[64:128, p, :],
                                        rinv[64:128, p:p + 1])
            nc.tensor.transpose(trs[:, i * P:(i + 1) * P], sblk, ident16)
        nc.scalar.copy(out=sblkT[:, sg * 512:(sg + 1) * 512], in_=trs)

    pclose(tr_ps_cm)

    tail_cm, tail_pools = zip(*[
        popen("wout_pool", 1), popen("w2sb_pool", 1), popen("w3sb_pool", 1),
        popen("finm_psum", 6, "PSUM"), popen("xrpool", 3), popen("outpool", 3),
    ])
    wout_pool, w2sb_pool, w3sb_pool, finm_psum, xrpool, outpool = tail_pools

    wout = [wout_pool.tile([P, D], bf16, tag=f"wo{i}", name=f"wo{i}")
            for i in range(NC_)]
    for ct in range(NC_):
        nc.sync.dma_start(out=wout[ct], in_=wout_ap[ct * P:(ct + 1) * P, :])

    # W2 = blockdiag(s) @ Wout; pair p's rows live in wout tile p.
    w2_sb = [w2sb_pool.tile([P, D], bf16, tag=f"w2_{i}", name=f"w2_{i}")
             for i in range(NC_)]
    for p in range(NC_):
        mp0 = finm_psum.tile([P, 512], f32, tag="fm", name="mp0")
        mp1 = finm_psum.tile([P, 512], f32, tag="fm", name="mp1")
        sl = sblkT[:, p * P:(p + 1) * P]
        nc.tensor.matmul(mp0, sl, wout[p][:, 0:512], start=True, stop=True)
        nc.tensor.matmul(mp1, sl, wout[p][:, 512:1024], start=True, stop=True)
        nc.vector.tensor_copy(out=w2_sb[p][:, 0:512], in_=mp0)
        nc.scalar.copy(out=w2_sb[p][:, 512:1024], in_=mp1)

    # W3 = Wq @ W2  (wqT holds Wq^T so hd is the contraction/partition dim).
    w3_sb = [w3sb_pool.tile([P, D], bf16, tag=f"w3_{i}", name=f"w3_{i}")
             for i in range(NC_)]
    for cb in range(NC_):
        mp0 = finm_psum.tile([P, 512], f32, tag="fm", name="mp0")
        mp1 = finm_psum.tile([P, 512], f32, tag="fm", name="mp1")
        for pt in range(NC_):
            lhs = wqT[pt][:, cb * P:(cb + 1) * P]
            nc.tensor.matmul(mp0, lhs, w2_sb[pt][:, 0:512],
                             start=(pt == 0), stop=(pt == NC_ - 1))
            nc.tensor.matmul(mp1, lhs, w2_sb[pt][:, 512:1024],
                             start=(pt == 0), stop=(pt == NC_ - 1))
        nc.vector.tensor_copy(out=w3_sb[cb][:, 0:512], in_=mp0)
        nc.scalar.copy(out=w3_sb[cb][:, 512:1024], in_=mp1)

    # ---- Phase D: out = xn @ W3 + x ----
    for tt in range(NT):
        tsl = slice(tt * P, (tt + 1) * P)
        xr = xrpool.tile([P, D], f32, tag="xr", name="xr")
        nc.sync.dma_start(out=xr, in_=x_ap[tsl, :])
        out_t = outpool.tile([P, D], f32, tag="out", name="out_t")
        mp0 = finm_psum.tile([P, 512], f32, tag="fm", name="mp0")
        mp1 = finm_psum.tile([P, 512], f32, tag="fm", name="mp1")
        for ct in range(NC_):
            lhs = xnT[ct][:, tsl]
            nc.tensor.matmul(mp0, lhs, w3_sb[ct][:, 0:512],
                             start=(ct == 0), stop=(ct == NC_ - 1))
            nc.tensor.matmul(mp1, lhs, w3_sb[ct][:, 512:1024],
                             start=(ct == 0), stop=(ct == NC_ - 1))
        nc.vector.tensor_add(out=out_t[:, 0:512], in0=mp0, in1=xr[:, 0:512])
        nc.vector.tensor_add(out=out_t[:, 512:1024], in0=mp1, in1=xr[:, 512:1024])
        nc.sync.dma_start(out=out_ap[tsl, :], in_=out_t)

    for cm in reversed(tail_cm):
        pclose(cm)
    for cm in (wqT_cm, xnT_cm, xn_cm, dram_cm, smpool_cm,
               consts_cm):
        pclose(cm)


def _make_program():
    """Build and compile the SPMD Bass program once."""
    import concourse.bass as bass  # noqa: F401
    import concourse.tile as tile
    from concourse import bacc, mybir

    nc = bacc.Bacc("TRN2", target_bir_lowering=False, debug=False,
                   num_devices=NCORES)
    f32 = mybir.dt.float32
    bf16 = mybir.dt.bfloat16
    x_d = nc.dram_tensor("x_shard", [TOK, D], f32, kind="ExternalInput").ap()
    wk_d = nc.dram_tensor("w_k", [D, D], f32, kind="ExternalInput").ap()
    wv_d = nc.dram_tensor("w_v", [D, D], f32, kind="ExternalInput").ap()
    wqT_d = nc.dram_tensor("w_qT", [D, D], bf16, kind="ExternalInput").ap()
    wout_d = nc.dram_tensor("w_out", [D, D], bf16, kind="ExternalInput").ap()
    out_d = nc.dram_tensor("out_shard", [TOK, D], f32, kind="ExternalOutput").ap()

    with tile.TileContext(nc) as tc:
        _build(tc, nc, mybir, x_d, wk_d, wv_d, wqT_d, wout_d, out_d)
    nc.compile()
    return nc


_CACHED_NC = None


def _prepare_inputs(x, w_qkv, b_qkv, w_out, b_out, ln_g, ln_b):
    import ml_dtypes

    bf16 = ml_dtypes.bfloat16
    x = np.ascontiguousarray(np.asarray(x, dtype=np.float32))
    w_qkv = np.asarray(w_qkv, dtype=np.float32)
    b_qkv = np.asarray(b_qkv, dtype=np.float32)
    w_out = np.asarray(w_out, dtype=np.float32)
    b_out = np.asarray(b_out, dtype=np.float32)
    ln_g = np.asarray(ln_g, dtype=np.float32)
    ln_b = np.asarray(ln_b, dtype=np.float32)

    # Fold the LN affine into the QKV projection: xn@W + b with xn = z*g + lb
    # becomes z@(g[:,None]*W) + (b + lb@W).
    w_f = ln_g[:, None] * w_qkv
    b_eff = b_qkv + ln_b @ w_qkv
    if np.abs(b_eff).max() > 0 or np.abs(b_out).max() > 0:
        raise NotImplementedError("nonzero effective biases not supported")

    wqT = np.ascontiguousarray(w_f[:, 0:D].T).astype(bf16)
    wk = np.ascontiguousarray(w_f[:, D:2 * D])
    wv = np.ascontiguousarray(w_f[:, 2 * D:3 * D])
    wout = np.ascontiguousarray(w_out).astype(bf16)

    shards = x.reshape(NCORES, TOK, D)
    in_maps = [
        {"x_shard": np.ascontiguousarray(shards[c]), "w_k": wk,
         "w_v": wv, "w_qT": wqT, "w_out": wout}
        for c in range(NCORES)
    ]
    return in_maps


def _run(inputs, trace=False):
    global _CACHED_NC
    from concourse.bass_utils import run_bass_kernel_spmd

    in_maps = _prepare_inputs(**inputs)
    if _CACHED_NC is None:
        _CACHED_NC = _make_program()
    res = run_bass_kernel_spmd(
        _CACHED_NC, in_maps, core_ids=list(range(NCORES)), trace=trace,
    )
    out = np.empty((B, L, D), dtype=np.float32)
    flat = out.reshape(NCORES, TOK, D)
    for c in range(NCORES):
        flat[c] = res.results[c]["out_shard"]
    return out, res


def kernel(**inputs):
    out, _ = _run(inputs, trace=False)
    return out


# revision 20
# speedup vs baseline: 1.4273x; 1.0723x over previous
"""Trainium2 Bass kernel for fused LN -> QKV -> (K^T V softmax) linear-attention -> out-proj + residual.

Algebraic restructure vs the direct formulation (kt_v is per-head 64x64 over
the whole sequence, so the K/V path funnels through small weight-side
products):

    xn   = (x - mu) / sigma                    (ln_g folded into the weights)
    G    = xn^T xn                             (1024x1024 Gram, contract tokens)
    ktv  = Wk^T G Wv        per head h: ktv_h = Wk_h^T G Wv_h    (linear in G)
    s    = softmax(ktv, axis=e)
    out  = (xn Wq) @ (blockdiag(s) Wout) + x   (q @ W2; q hides the AllReduce)

K and V activations are never materialized: G costs half the K,V projection
and ktv is tiny. The Q projection is kept (rather than folding Wq into a
single W3 = Wq S Wout) because it is collective-independent: issued right
after the ktv AllReduce starts, its ~55us of PE work fully hides the
collective's ~50us latency.

Precision split (the softmax logits are hypersensitive -- std ~64 -- so the
ktv path must stay f32): xn/G/A/Wk/Wv are f32r (1 cycle/row on the PE for
free-dim >= 256, same as bf16); the post-softmax chain (s, Wout, W2) and the
q-path (xnT, Wq, qT) are bf16 (~2e-3 effect each).

Sharding: data-parallel, 8 shards of 2048 tokens. Cores 2b, 2b+1 hold batch
element b; ktv partials (linear in G) are AllReduced pairwise, everything
else is local.
"""

import numpy as np

# Problem shapes (hardcoded per harness contract).
B, L, D = 4, 4096, 1024
H, HD = 16, 64
NCORES = 8
TOK = B * L // NCORES  # 2048 tokens per core
P = 128
NT = TOK // P  # 16 token tiles per core
NC_ = D // P  # 8 channel tiles
EPS = 1e-5


def _build(tc, nc, mybir, x_ap, wk_ap, wv_ap, wqT_ap, wout_ap, out_ap,
           use_collective=True):
    from concourse.masks import make_identity

    f32 = mybir.dt.float32
    f32r = mybir.dt.float32r
    bf16 = mybir.dt.bfloat16
    AF = mybir.ActivationFunctionType
    OP = mybir.AluOpType

    def popen(name, bufs, space="SBUF"):
        cm = tc.tile_pool(name=name, bufs=bufs, space=space)
        return cm, cm.__enter__()

    def pclose(cm):
        cm.__exit__(None, None, None)

    consts_cm, consts = popen("consts", 1)
    smpool_cm, smpool = popen("smpool", 1)
    dram_cm, dram = popen("dram", 1, space="DRAM")
    xn_cm, xn_pool = popen("xn_pool", 1)

    ident32 = consts.tile([P, P], f32)
    make_identity(nc, ident32)
    identr = consts.tile([P, P], f32r)
    nc.vector.tensor_copy(out=identr, in_=ident32)
    ident16 = consts.tile([P, P], bf16)
    make_identity(nc, ident16)
    eps_t = consts.tile([P, 1], f32)
    nc.vector.memset(eps_t, EPS)
    zero_t = consts.tile([P, P], bf16)
    nc.vector.memset(zero_t, 0.0)

    # Warm-up collective: absorbs the mesh/staging setup cost so the real
    # ktv AllReduce later starts hot.
    warm_sb = consts.tile([P, 4], f32)
    nc.vector.memset(warm_sb, 0.0)
    warm_in = dram.tile([P, 4], f32, name="warm_in")
    warm_out = dram.tile([P, 4], f32, name="warm_out")
    nc.gpsimd.dma_start(out=warm_in, in_=warm_sb)
    if use_collective:
        nc.gpsimd.collective_compute(
            "AllReduce",
            mybir.AluOpType.add,
            ins=[warm_in.opt()],
            outs=[warm_out.opt()],
            replica_groups=[[0, 1], [2, 3], [4, 5], [6, 7]],
        )

    # Persistent f32r xn (token-major) for the Gram path.
    xn = [xn_pool.tile([P, D], f32r, tag=f"xn{i}", name=f"xn{i}")
          for i in range(NT)]

    # f32 K/V projection weights and the Gram matrix (both freed at the
    # collective kick, so the bf16 tail reuses their SBUF space).
    # DMA queue split: x tiles stream on sync; wk on vector, wv on scalar so
    # the first LN tiles are not stuck behind 16MB of weight descriptors.
    wkv_cm, wkv_pool = popen("wkv_pool", 1)
    gsb_cm, gsb_pool = popen("gsb_pool", 1)
    g_sb = [gsb_pool.tile([P, D], f32r, tag=f"g{i}", name=f"gsb{i}")
            for i in range(NC_)]
    wk = [wkv_pool.tile([P, D], f32r, tag=f"wk{i}", name=f"wk{i}")
          for i in range(NC_)]
    wv = [wkv_pool.tile([P, D], f32r, tag=f"wv{i}", name=f"wv{i}")
          for i in range(NC_)]
    for ct in range(NC_):
        nc.gpsimd.dma_start(out=wk[ct], in_=wk_ap[ct * P:(ct + 1) * P, :].bitcast(f32r))
        nc.gpsimd.dma_start(out=wv[ct], in_=wv_ap[ct * P:(ct + 1) * P, :].bitcast(f32r))

    # ---- Phase A: LN -> xn (f32) + Gram accumulation ----
    # G needs 16 PSUM banks; ping-pong two 3-bank tag sets (6 banks + mm's 2)
    # across 6 passes over the resident xn tiles so pass N+1 never waits on
    # pass N's drains.
    a_cm, a_pools = zip(*[popen("xpool", 3), popen("stpool", 4)])
    xpool, stpool = a_pools
    g_ps_cm, g_psum = popen("g_psum", 1, space="PSUM")

    def ln_tile(tt):
        x_t = xpool.tile([P, D], f32, tag="x", name="x_t")
        nc.sync.dma_start(out=x_t, in_=x_ap[tt * P:(tt + 1) * P, :])
        stats = stpool.tile([P, 2, 6], f32, tag="stats", name="stats")
        nc.vector.bn_stats(out=stats[:, 0, :], in_=x_t[:, 0:512])
        nc.vector.bn_stats(out=stats[:, 1, :], in_=x_t[:, 512:1024])
        mv = stpool.tile([P, 2], f32, tag="mv", name="mv")
        nc.vector.bn_aggr(out=mv, in_=stats)
        sd = stpool.tile([P, 1], f32, tag="sd", name="sd")
        nc.scalar.activation(out=sd, in_=mv[:, 1:2], func=AF.Sqrt, bias=eps_t)
        rstd = stpool.tile([P, 1], f32, tag="rstd", name="rstd")
        nc.vector.reciprocal(out=rstd, in_=sd)
        nmr = stpool.tile([P, 1], f32, tag="nmr", name="nmr")
        nc.vector.tensor_scalar(out=nmr, in0=mv[:, 0:1], scalar1=rstd,
                                scalar2=-1.0, op0=OP.mult, op1=OP.mult)
        nc.scalar.activation(out=xn[tt], in_=x_t, func=AF.Identity, scale=rstd,
                             bias=nmr)

    # G is symmetric: compute the top 4 block-rows in full (8 half-blocks,
    # all 8 PSUM banks, overlapped with LN) plus the diagonal quarter
    # (cb 4-7, right half), then mirror the lower-left quarter by
    # PE-transposing G[0:4, 512:1024].
    GROUPS = [[(cb, h) for cb in range(4) for h in range(2)],
              [(cb, 1) for cb in range(4, 8)]]
    for pi, group in enumerate(GROUPS):
        ps = {}
        for si, (cb, h) in enumerate(group):
            slot = si + 4 if pi == 1 else si
            ps[(cb, h)] = g_psum.tile([P, 512], f32, tag=f"gp{slot}",
                                      name=f"gps{cb}_{h}")
        for tt in range(NT):
            if pi == 0:
                ln_tile(tt)
            for (cb, h) in group:
                nc.tensor.matmul(ps[(cb, h)], xn[tt][:, cb * P:(cb + 1) * P],
                                 xn[tt][:, h * 512:(h + 1) * 512],
                                 start=(tt == 0), stop=(tt == NT - 1))
        keys = list(ps)
        if pi == 0:
            keys = keys[4:] + keys[:4]  # drain pass-2's reuse slots first
        for i, (cb, h) in enumerate(keys):
            if i % 2 == 0:
                nc.vector.tensor_copy(out=g_sb[cb][:, h * 512:(h + 1) * 512],
                                      in_=ps[(cb, h)])
            else:
                nc.scalar.copy(out=g_sb[cb][:, h * 512:(h + 1) * 512],
                               in_=ps[(cb, h)])

    pclose(g_ps_cm)
    for cm in reversed(a_cm):
        pclose(cm)

    # Mirror: g_sb[4+i][:, 0:512] (4 blocks each) = G[0:4, 512:1024]^T.
    trm_cm, trm_psum = popen("trm_psum", 2, space="PSUM")
    for i in range(4):
        trm = trm_psum.tile([P, 512], f32r, tag="trm", name="trm")
        for j in range(4):
            nc.tensor.transpose(
                trm[:, j * P:(j + 1) * P],
                g_sb[j][:, 512 + i * P:512 + (i + 1) * P], identr)
        if i % 2 == 0:
            nc.vector.tensor_copy(out=g_sb[4 + i][:, 0:512], in_=trm)
        else:
            nc.scalar.copy(out=g_sb[4 + i][:, 0:512], in_=trm)
    pclose(trm_cm)

    # ---- Phase A3 (fused): per c-block, A[cb] = G[:,cb]^T-chain @ Wv, then
    # immediately fold A[cb] into all 8 ktv pair-chains (contraction over cb).
    asb_cm, asb_pool = popen("asb_pool", 3)
    ktv_ps_cm, ktv_psum = popen("ktv_psum", 1, space="PSUM")
    a3m_cm, a3m_psum = popen("a3m_psum", 2, space="PSUM")
    a3x_cm, a3x_psum = popen("a3x_psum", 2, space="PSUM")

    # Two head pairs share one [P, 512] PSUM bank; only the very first matmul
    # into a bank sets start=True (marks the whole bank pending-zero, so the
    # second pair's first write is zero-initialized by the hardware).
    kt = [ktv_psum.tile([P, 512], f32, tag=f"kt{q}", name=f"kt{q}")
          for q in range(4)]

    def ktv_mm(cb, i):
        # i-th of the 8 ktv matmuls folding a_t[cb] into the pair chains.
        q, pr = i // 2, i % 2
        p = 2 * q + pr
        nc.tensor.matmul(kt[q][:, pr * 256:(pr + 1) * 256],
                         wk[cb][:, p * P:(p + 1) * P],
                         a_sb[cb][:, (p // 2) * 256:(p // 2 + 1) * 256],
                         start=(cb == 0 and pr == 0),
                         stop=(cb == NC_ - 1 and pr == 1))

    # ktv(cb-1)'s 8 short LDWEIGHTS-bound matmuls are interleaved into
    # A-chain(cb)'s 16 long ones so their weight loads hide.
    a_sb = {}
    for cb in range(NC_ + 1):
        if cb < NC_:
            mp0 = a3m_psum.tile([P, 512], f32, tag="mma", name="mp0")
            mp1 = a3x_psum.tile([P, 512], f32, tag="mmx", name="mp1")
            for ct in range(NC_):
                lhs = g_sb[ct][:, cb * P:(cb + 1) * P]
                nc.tensor.matmul(mp0, lhs, wv[ct][:, 0:512],
                                 start=(ct == 0), stop=(ct == NC_ - 1))
                if cb >= 1:
                    ktv_mm(cb - 1, ct)
                nc.tensor.matmul(mp1, lhs, wv[ct][:, 512:1024],
                                 start=(ct == 0), stop=(ct == NC_ - 1))
            a_t = asb_pool.tile([P, D], f32r, tag="a", name="a_t")
            nc.vector.tensor_copy(out=a_t[:, 0:512], in_=mp0)
            nc.scalar.copy(out=a_t[:, 512:1024], in_=mp1)
            a_sb[cb] = a_t
        else:
            for i in range(8):
                ktv_mm(NC_ - 1, i)

    # Stage ktv diag blocks: partition (h%2)*64+d, free (g=h//2, e).
    # Pair p sits in bank q=p//2 at column base (p%2)*256; within its
    # 256-wide quad slice head 2p is at offset (p%2)*128.
    stage = smpool.tile([P, 8, 64], f32, tag="sm864", name="stage")
    for p in range(NC_):
        q, pr = p // 2, p % 2
        off = pr * 256 + pr * 128
        nc.vector.tensor_copy(out=stage[0:64, p, :],
                              in_=kt[q][0:64, off:off + 64])
        nc.scalar.copy(out=stage[64:128, p, :],
                       in_=kt[q][64:128, off + 64:off + 128])

    # ---- Phase B: AllReduce ktv partials across the batch pair ----
    bounce_in = dram.tile([P, 512], f32, name="bounce_in")
    bounce_out = dram.tile([P, 512], f32, name="bounce_out")
    nc.gpsimd.dma_start(out=bounce_in, in_=stage.rearrange("p g e -> p (g e)"))
    if use_collective:
        nc.gpsimd.collective_compute(
            "AllReduce",
            mybir.AluOpType.add,
            ins=[bounce_in.opt()],
            outs=[bounce_out.opt()],
            replica_groups=[[0, 1], [2, 3], [4, 5], [6, 7]],
        )
    else:
        nc.gpsimd.dma_start(out=bounce_out, in_=bounce_in)
    kv_red = smpool.tile([P, 8, 64], f32, name="kv_red")
    nc.gpsimd.dma_start(out=kv_red.rearrange("p g e -> p (g e)"), in_=bounce_out)

    pclose(a3x_cm)
    pclose(a3m_cm)
    pclose(ktv_ps_cm)
    pclose(asb_cm)
    pclose(gsb_cm)
    pclose(wkv_cm)

    # ---- Transposes xn -> xnT (hide the collective) ----
    xnT_cm, xnT_pool = popen("xnT_pool", 1)
    wqT_cm, wqT_pool = popen("wqT_pool", 1)
    tr_ps_cm, tr_psum = popen("tr_psum", 2, space="PSUM")

    xnT = [xnT_pool.tile([P, TOK], bf16, tag=f"xnT{i}", name=f"xnT{i}")
           for i in range(NC_)]
    wqT = [wqT_pool.tile([P, D], bf16, tag=f"wqT{i}", name=f"wqT{i}")
           for i in range(NC_)]
    for ct in range(NC_):
        nc.sync.dma_start(out=wqT[ct], in_=wqT_ap[ct * P:(ct + 1) * P, :])

    for ct in range(NC_):
        for tg in range(NT // 4):
            trt = tr_psum.tile([P, 512], f32r, tag="tr", name="trt")
            for i in range(4):
                tt = tg * 4 + i
                nc.tensor.transpose(trt[:, i * P:(i + 1) * P],
                                    xn[tt][:, ct * P:(ct + 1) * P], identr)
            if tg % 2 == 0:
                nc.vector.tensor_copy(out=xnT[ct][:, tg * 512:(tg + 1) * 512],
                                      in_=trt)
            else:
                nc.scalar.copy(out=xnT[ct][:, tg * 512:(tg + 1) * 512], in_=trt)

    # ---- Phase C: softmax -> s^T (block-diag pairs) -> W2 -> W3 ----
    negmax = smpool.tile([P, 8], f32, name="negmax")
    nc.vector.reduce_max(out=negmax, in_=kv_red, axis=mybir.AxisListType.X,
                         negate=True)
    s_t = smpool.tile([P, 8, 64], f32, tag="sm864b", name="s_t")
    sums = smpool.tile([P, 8], f32, name="sums")
    for g in range(8):
        nc.scalar.activation(out=s_t[:, g, :], in_=kv_red[:, g, :],
                             func=AF.Exp, bias=negmax[:, g:g + 1],
                             accum_out=sums[:, g:g + 1])
    rinv = smpool.tile([P, 8], f32, name="rinv")
    nc.vector.reciprocal(out=rinv, in_=sums)

    sblkT = smpool.tile([P, NC_ * P], bf16, tag="sbT", name="sblkT")
    for sg in range(2):
        trs = tr_psum.tile([P, 512], bf16, tag="trs", name="trs")
        for i in range(4):
            p = sg * 4 + i
            sblk = smpool.tile([P, P], bf16, tag=f"sbk{p % 2}", name="sblk")
            nc.vector.tensor_copy(out=sblk, in_=zero_t)
            nc.vector.tensor_scalar_mul(sblk[0:64, 0:64], s_t[0:64, p, :],
                                        rinv[0:64, p:p + 1])
            nc.vector.tensor_scalar_mul(sblk[64:128, 64:128], s_t[64:128, p, :],
                                        rinv[64:128, p:p + 1])
            nc.tensor.transpose(trs[:, i * P:(i + 1) * P], sblk, ident16)
        nc.scalar.copy(out=sblkT[:, sg * 512:(sg + 1) * 512], in_=trs)

    pclose(tr_ps_cm)

    tail_cm, tail_pools = zip(*[
        popen("wout_pool", 1), popen("w2sb_pool", 1), popen("w3sb_pool", 1),
        popen("finm_psum", 6, "PSUM"), popen("xrpool", 3), popen("outpool", 3),
    ])
    wout_pool, w2sb_pool, w3sb_pool, finm_psum, xrpool, outpool = tail_pools

    wout = [wout_pool.tile([P, D], bf16, tag=f"wo{i}", name=f"wo{i}")
            for i in range(NC_)]
    for ct in range(NC_):
        nc.sync.dma_start(out=wout[ct], in_=wout_ap[ct * P:(ct + 1) * P, :])

    # W2 = blockdiag(s) @ Wout; pair p's rows live in wout tile p.
    w2_sb = [w2sb_pool.tile([P, D], bf16, tag=f"w2_{i}", name=f"w2_{i}")
             for i in range(NC_)]
    for p in range(NC_):
        mp0 = finm_psum.tile([P, 512], f32, tag="fm", name="mp0")
        mp1 = finm_psum.tile([P, 512], f32, tag="fm", name="mp1")
        sl = sblkT[:, p * P:(p + 1) * P]
        nc.tensor.matmul(mp0, sl, wout[p][:, 0:512], start=True, stop=True)
        nc.tensor.matmul(mp1, sl, wout[p][:, 512:1024], start=True, stop=True)
        nc.vector.tensor_copy(out=w2_sb[p][:, 0:512], in_=mp0)
        nc.scalar.copy(out=w2_sb[p][:, 512:1024], in_=mp1)

    # W3 = Wq @ W2  (wqT holds Wq^T so hd is the contraction/partition dim).
    w3_sb = [w3sb_pool.tile([P, D], bf16, tag=f"w3_{i}", name=f"w3_{i}")
             for i in range(NC_)]
    for cb in range(NC_):
        mp0 = finm_psum.tile([P, 512], f32, tag="fm", name="mp0")
        mp1 = finm_psum.tile([P, 512], f32, tag="fm", name="mp1")
        for pt in range(NC_):
            lhs = wqT[pt][:, cb * P:(cb + 1) * P]
            nc.tensor.matmul(mp0, lhs, w2_sb[pt][:, 0:512],
                             start=(pt == 0), stop=(pt == NC_ - 1))
            nc.tensor.matmul(mp1, lhs, w2_sb[pt][:, 512:1024],
                             start=(pt == 0), stop=(pt == NC_ - 1))
        nc.vector.tensor_copy(out=w3_sb[cb][:, 0:512], in_=mp0)
        nc.scalar.copy(out=w3_sb[cb][:, 512:1024], in_=mp1)

    # ---- Phase D: out = xn @ W3 + x ----
    for tt in range(NT):
        tsl = slice(tt * P, (tt + 1) * P)
        xr = xrpool.tile([P, D], f32, tag="xr", name="xr")
        nc.sync.dma_start(out=xr, in_=x_ap[tsl, :])
        out_t = outpool.tile([P, D], f32, tag="out", name="out_t")
        mp0 = finm_psum.tile([P, 512], f32, tag="fm", name="mp0")
        mp1 = finm_psum.tile([P, 512], f32, tag="fm", name="mp1")
        for ct in range(NC_):
            lhs = xnT[ct][:, tsl]
            nc.tensor.matmul(mp0, lhs, w3_sb[ct][:, 0:512],
                             start=(ct == 0), stop=(ct == NC_ - 1))
            nc.tensor.matmul(mp1, lhs, w3_sb[ct][:, 512:1024],
                             start=(ct == 0), stop=(ct == NC_ - 1))
        nc.vector.tensor_add(out=out_t[:, 0:512], in0=mp0, in1=xr[:, 0:512])
        nc.vector.tensor_add(out=out_t[:, 512:1024], in0=mp1, in1=xr[:, 512:1024])
        nc.sync.dma_start(out=out_ap[tsl, :], in_=out_t)

    for cm in reversed(tail_cm):
        pclose(cm)
    for cm in (wqT_cm, xnT_cm, xn_cm, dram_cm, smpool_cm,
               consts_cm):
        pclose(cm)


def _make_program():
    """Build and compile the SPMD Bass program once."""
    import concourse.bass as bass  # noqa: F401
    import concourse.tile as tile
    from concourse import bacc, mybir

    nc = bacc.Bacc("TRN2", target_bir_lowering=False, debug=False,
                   num_devices=NCORES)
    f32 = mybir.dt.float32
    bf16 = mybir.dt.bfloat16
    x_d = nc.dram_tensor("x_shard", [TOK, D], f32, kind="ExternalInput").ap()
    wk_d = nc.dram_tensor("w_k", [D, D], f32, kind="ExternalInput").ap()
    wv_d = nc.dram_tensor("w_v", [D, D], f32, kind="ExternalInput").ap()
    wqT_d = nc.dram_tensor("w_qT", [D, D], bf16, kind="ExternalInput").ap()
    wout_d = nc.dram_tensor("w_out", [D, D], bf16, kind="ExternalInput").ap()
    out_d = nc.dram_tensor("out_shard", [TOK, D], f32, kind="ExternalOutput").ap()

    with tile.TileContext(nc) as tc:
        _build(tc, nc, mybir, x_d, wk_d, wv_d, wqT_d, wout_d, out_d)
    nc.compile()
    return nc


_CACHED_NC = None


def _prepare_inputs(x, w_qkv, b_qkv, w_out, b_out, ln_g, ln_b):
    import ml_dtypes

    bf16 = ml_dtypes.bfloat16
    x = np.ascontiguousarray(np.asarray(x, dtype=np.float32))
    w_qkv = np.asarray(w_qkv, dtype=np.float32)
    b_qkv = np.asarray(b_qkv, dtype=np.float32)
    w_out = np.asarray(w_out, dtype=np.float32)
    b_out = np.asarray(b_out, dtype=np.float32)
    ln_g = np.asarray(ln_g, dtype=np.float32)
    ln_b = np.asarray(ln_b, dtype=np.float32)

    # Fold the LN affine into the QKV projection: xn@W + b with xn = z*g + lb
    # becomes z@(g[:,None]*W) + (b + lb@W).
    w_f = ln_g[:, None] * w_qkv
    b_eff = b_qkv + ln_b @ w_qkv
    if np.abs(b_eff).max() > 0 or np.abs(b_out).max() > 0:
        raise NotImplementedError("nonzero effective biases not supported")

    wqT = np.ascontiguousarray(w_f[:, 0:D].T).astype(bf16)
    wk = np.ascontiguousarray(w_f[:, D:2 * D])
    wv = np.ascontiguousarray(w_f[:, 2 * D:3 * D])
    wout = np.ascontiguousarray(w_out).astype(bf16)

    shards = x.reshape(NCORES, TOK, D)
    in_maps = [
        {"x_shard": np.ascontiguousarray(shards[c]), "w_k": wk,
         "w_v": wv, "w_qT": wqT, "w_out": wout}
        for c in range(NCORES)
    ]
    return in_maps


def _run(inputs, trace=False):
    global _CACHED_NC
    from concourse.bass_utils import run_bass_kernel_spmd

    in_maps = _prepare_inputs(**inputs)
    if _CACHED_NC is None:
        _CACHED_NC = _make_program()
    res = run_bass_kernel_spmd(
        _CACHED_NC, in_maps, core_ids=list(range(NCORES)), trace=trace,
    )
    out = np.empty((B, L, D), dtype=np.float32)
    flat = out.reshape(NCORES, TOK, D)
    for c in range(NCORES):
        flat[c] = res.results[c]["out_shard"]
    return out, res


def kernel(**inputs):
    out, _ = _run(inputs, trace=False)
    return out
